# revision 20
# baseline (speedup 1.0000x reference)
"""Multi-head attention (B=1, S=4096, D=1024, H=16) on 8 TRN2 NeuronCores.

Sharding: tensor-parallel over heads (2 heads/core) for QKV+attention, then
an AllToAll redistributes normalized attn^T so each core owns S/8 query rows
across ALL heads and applies the full Wo locally (no ReduceScatter of fp32
partials; collective traffic drops from ~14MB to 256KB per core).

v3 pipeline (ACT-bound design, exp = 33.5M elem/core ~ 294us floor):
- phase 1: xT DMA (t-tile-paced) -> K^T proj (weights stationary, 8 psum
  banks) -> Q^T group 0. V is computed as V^T (N=512 matmuls) and converted
  to natural layout with DMA-transposes, injected into early phase-2 slack.
- phase 2: linear step loop m over (qc, i-half, kt): scores emitted 1 step
  ahead (s_ps double-buffered [128,1024] = both heads for 512 q), exp runs
  back-to-back on the scalar engine, attnV (M=65, fused softmax-denominator
  ones column) lags LAG steps behind. Remaining Q groups / V^T groups /
  out-proj work are drip-injected into PE slack.
- per (qc,i): denominators -> reciprocal -> one K=2 fp32 matmul broadcasts
  both heads' recips across partitions -> bf16 normalized attn^T -> DMA to
  the AllToAll staging buffer. Per qc: AllToAll [8,128,128] bf16 (64KB),
  out-proj (K=1024, full Wo^T) injected into the next qc's steps.
"""

import sys

sys.path.insert(0, "/opt/trn_rl_repo")

from collections import deque

import ml_dtypes
import numpy as np

import concourse.bass as bass
import concourse.mybir as mybir
import concourse.tile as tile
from concourse import bacc
from concourse.bass_utils import run_bass_kernel_spmd

N_CORES = 8
S = 4096
D = 1024
H = 16
DK = 64
DH = 128  # head-dims per core (2 heads x 64)
NKT = S // 128  # 32 key tiles
N_QC = 4  # output chunks (1024 q rows each)
F32 = mybir.dt.float32
BF16 = mybir.dt.bfloat16
NP_BF16 = ml_dtypes.bfloat16
Exp = mybir.ActivationFunctionType.Exp


def _build(with_bias=False, with_mask=False):
    nc = bacc.Bacc("TRN2", target_bir_lowering=False, debug=False, num_devices=N_CORES)

    xT = nc.dram_tensor("xT", [D, S], BF16, kind="ExternalInput")
    wqT = nc.dram_tensor("wqT", [D, DH], BF16, kind="ExternalInput")
    wkT = nc.dram_tensor("wkT", [D, DH], BF16, kind="ExternalInput")
    wvT = nc.dram_tensor("wvT", [D, DH], BF16, kind="ExternalInput")
    woT = nc.dram_tensor("woT", [DH, D], BF16, kind="ExternalInput")
    bq = nc.dram_tensor("bq", [1, DH], BF16, kind="ExternalInput")
    bk = nc.dram_tensor("bk", [1, DH], BF16, kind="ExternalInput")
    bv = nc.dram_tensor("bv", [1, DH], BF16, kind="ExternalInput")
    bo = nc.dram_tensor("bo", [1, D], BF16, kind="ExternalInput")
    if with_mask:
        maskT = nc.dram_tensor("maskT", [128, NKT], BF16, kind="ExternalInput")
    out_ext = nc.dram_tensor("out", [N_QC, 1024 // N_CORES, D], F32, kind="ExternalOutput")

    DT = D // 128  # 8 contraction tiles
    N_STEP = N_QC * 2 * NKT  # 256 linear steps (qc, i, kt)

    with tile.TileContext(nc) as tc:
        with (
            tc.tile_pool(name="const", bufs=1) as const,
            tc.tile_pool(name="main", bufs=1) as main,
            tc.tile_pool(name="pp", bufs=15) as pp,
            tc.tile_pool(name="atp", bufs=2) as atp,
            tc.tile_pool(name="oup", bufs=2) as oup,
            tc.tile_pool(name="rbp", bufs=2) as rbp,
            tc.tile_pool(name="stp", bufs=3) as stp,
            tc.tile_pool(name="dram", bufs=1, space="DRAM") as dram,
        ):
            # ---- constants ----
            wq_sb = const.tile([128, DT, DH], BF16, tag="wq")
            wk_sb = const.tile([128, DT, DH], BF16, tag="wk")
            wv_sb = const.tile([128, DT, DH], BF16, tag="wv")
            woT_sb = const.tile([DH, D], BF16, tag="wo")
            nc.gpsimd.dma_start(woT_sb[:], woT[:, :])
            for t in range(DT):
                tsl = slice(t * 128, (t + 1) * 128)
                nc.scalar.dma_start(wk_sb[:, t, :], wkT[tsl, :])
                nc.sync.dma_start(wq_sb[:, t, :], wqT[tsl, :])
                nc.scalar.dma_start(wv_sb[:, t, :], wvT[tsl, :])
            ones64 = const.tile([1, DK], BF16, tag="ones64")
            nc.vector.memset(ones64[:], 1.0)
            den0_sb = const.tile([1, 512], F32, tag="den0")
            den1_sb = const.tile([1, 512], F32, tag="den1")
            rec0_sb = const.tile([1, 512], F32, tag="rec0")
            rec1_sb = const.tile([1, 512], F32, tag="rec1")
            rec0b_sb = const.tile([1, 512], BF16, tag="rec0b")
            rec1b_sb = const.tile([1, 512], BF16, tag="rec1b")
            ones_sb = const.tile([1, 512], BF16, tag="ones")
            nc.vector.memset(ones_sb[:], 1.0)
            if with_bias:
                bq_sb = const.tile([1, DH], BF16, tag="bq")
                bk_sb = const.tile([1, DH], BF16, tag="bk")
                bv_sb = const.tile([1, DH], BF16, tag="bv")
                bo_sb = const.tile([1, D], BF16, tag="bo")
                nc.sync.dma_start(bq_sb[:], bq[:, :])
                nc.sync.dma_start(bk_sb[:], bk[:, :])
                nc.sync.dma_start(bv_sb[:], bv[:, :])
                nc.sync.dma_start(bo_sb[:], bo[:, :])
            if with_mask:
                maskT_sb = const.tile([128, NKT], BF16, tag="maskT")
                nc.sync.dma_start(maskT_sb[:], maskT[:, :])

            # ---- big SBUF tensors ----
            xT_sb = main.tile([128, DT, S], BF16, tag="xt")
            QT_sb = main.tile([DH, S], BF16, tag="qt")
            KT_sb = main.tile([DH, S], BF16, tag="kt")
            VT_sb = main.tile([DH, S], BF16, tag="vt")
            # V natural per kt tile: [keys, kt, head, dk + ones col]
            vh_sb = main.tile([128, NKT, 2, DK + 1], BF16, tag="vh")
            nc.vector.memset(vh_sb[:, :, :, DK : DK + 1], 1.0)

            partial = [
                dram.tile([1024, D], BF16, name=f"partial{qc}")
                for qc in range(N_QC)
            ]
            rs_out = [
                dram.tile([1024 // N_CORES, D], BF16, name=f"rs_out{qc}")
                for qc in range(N_QC)
            ]

            _xq = {0: nc.sync, 1: nc.scalar, 2: nc.sync, 3: nc.scalar,
                   4: nc.sync, 5: nc.scalar, 6: nc.gpsimd, 7: nc.gpsimd}
            for t in range(DT):
                _xq[t].dma_start(xT_sb[:, t, :], xT[t * 128 : (t + 1) * 128, :])

            # ---- phase 1: K^T full + Q^T group 0 (8 psum banks) ----
            with tc.tile_pool(name="pj", bufs=1, space="PSUM") as pj:
                kps = [
                    pj.tile([128, 512], F32, tag=f"pj{g}", name=f"kps{g}")
                    for g in range(8)
                ]
                # HAM warmup: keep the PE busy while xT streams in so the
                # projection matmuls run at 2.4GHz instead of 1.2
                for w in range(16):
                    nc.tensor.matmul(
                        kps[7][:],
                        ones_sb[:, 0:128],
                        ones_sb[:],
                        start=True,
                        stop=True,
                    )
                for t in range(DT):
                    for g in range(8):
                        nc.tensor.matmul(
                            kps[g][:],
                            wk_sb[:, t, :],
                            xT_sb[:, t, g * 512 : (g + 1) * 512],
                            start=(t == 0),
                            stop=(t == DT - 1) and not with_bias,
                        )
                for g in range(8):
                    if with_bias:
                        nc.tensor.matmul(
                            kps[g][:], bk_sb[:], ones_sb[:], start=False, stop=True
                        )
                    nc.vector.tensor_copy(KT_sb[:, g * 512 : (g + 1) * 512], kps[g][:])
                # Q group 0 (reuse bank 0)
                qps0 = pj.tile([128, 512], F32, tag="pj0", name="qps0")
                for t in range(DT):
                    nc.tensor.matmul(
                        qps0[:],
                        wq_sb[:, t, :],
                        xT_sb[:, t, 0:512],
                        start=(t == 0),
                        stop=(t == DT - 1) and not with_bias,
                    )
                if with_bias:
                    nc.tensor.matmul(
                        qps0[:], bq_sb[:], ones_sb[:], start=False, stop=True
                    )
                nc.vector.tensor_copy(QT_sb[:, 0:512], qps0[:])

            # ---- phase 2 ----
            with (
                tc.tile_pool(name="scp", bufs=2, space="PSUM") as scp,
                tc.tile_pool(name="accp", bufs=1, space="PSUM") as accp,
                tc.tile_pool(name="mpp", bufs=2, space="PSUM") as mpp,
            ):
                # --- injected work: V^T groups + Q groups 1..7, out-proj ---
                inj = deque()

                def vt_group(g):
                    gsl = slice(g * 512, (g + 1) * 512)
                    cell = {}

                    def mm(t):
                        def run():
                            if t == 0:
                                cell["ps"] = mpp.tile(
                                    [128, 512], F32, tag="mp", name=f"vtp{g}"
                                )
                            nc.tensor.matmul(
                                cell["ps"][:],
                                wv_sb[:, t, :],
                                xT_sb[:, t, gsl],
                                start=(t == 0),
                                stop=(t == DT - 1) and not with_bias,
                            )

                        return run

                    def fin():
                        ps = cell["ps"]
                        if with_bias:
                            nc.tensor.matmul(
                                ps[:], bv_sb[:], ones_sb[:], start=False, stop=True
                            )
                        nc.vector.tensor_copy(VT_sb[:, gsl], ps[:])
                        for kt in range(4 * g, 4 * g + 4):
                            stage = stp.tile(
                                [128, 2, DK], BF16, tag="st", name=f"st{kt}"
                            )
                            nc.sync.dma_start(
                                stage[:],
                                VT_sb[:, kt * 128 : (kt + 1) * 128],
                                transpose=True,
                            )
                            nc.vector.tensor_copy(vh_sb[:, kt, :, 0:DK], stage[:])
                            if with_mask:
                                nc.vector.tensor_scalar_mul(
                                    vh_sb[:, kt, :, :],
                                    vh_sb[:, kt, :, :],
                                    maskT_sb[:, kt : kt + 1],
                                )

                    return [mm(t) for t in range(DT)] + [fin]

                def q_group(g):
                    gsl = slice(g * 512, (g + 1) * 512)
                    cell = {}

                    def mm(t):
                        def run():
                            if t == 0:
                                cell["ps"] = mpp.tile(
                                    [128, 512], F32, tag="mp", name=f"qp{g}"
                                )
                            nc.tensor.matmul(
                                cell["ps"][:],
                                wq_sb[:, t, :],
                                xT_sb[:, t, gsl],
                                start=(t == 0),
                                stop=(t == DT - 1) and not with_bias,
                            )

                        return run

                    def fin():
                        ps = cell["ps"]
                        if with_bias:
                            nc.tensor.matmul(
                                ps[:], bq_sb[:], ones_sb[:], start=False, stop=True
                            )
                        nc.vector.tensor_copy(QT_sb[:, gsl], ps[:])

                    return [mm(t) for t in range(DT)] + [fin]

                # priority: V^T groups early (attnV needs them), Q g1 before
                # step 32, Q g2-3 before step 64, rest before step 128.
                for g in (0, 1):
                    inj.extend(vt_group(g))
                inj.extend(q_group(1))
                for g in (2, 3):
                    inj.extend(vt_group(g))
                inj.extend(q_group(2))
                for g in (4, 5):
                    inj.extend(vt_group(g))
                inj.extend(q_group(3))
                for g in (6, 7):
                    inj.extend(vt_group(g))
                for g in (4, 5, 6, 7):
                    inj.extend(q_group(g))

                oinj = deque()  # out-projection + RS closures per (qc, i)

                def outproj(qc, i, aT):
                    def work(pt, dh):
                        def run():
                            dsl = slice(dh * 512, (dh + 1) * 512)
                            o = mpp.tile(
                                [128, 512], F32, tag="mp", name=f"o{qc}{i}{pt}{dh}"
                            )
                            nc.tensor.matmul(
                                o[:],
                                aT[:, pt * 128 : (pt + 1) * 128],
                                woT_sb[:, dsl],
                                start=True,
                                stop=not with_bias,
                            )
                            if with_bias:
                                nc.tensor.matmul(
                                    o[:],
                                    ones_sb[:, 0:128],
                                    bo_sb[:, dsl],
                                    start=False,
                                    stop=True,
                                )
                            osb = oup.tile([128, 512], BF16, tag="ou", name="osb")
                            nc.vector.tensor_copy(osb[:], o[:])
                            r0 = i * 512 + pt * 128
                            nc.sync.dma_start(
                                partial[qc][r0 : r0 + 128, dsl], osb[:]
                            )

                        return run

                    def rs():
                        nc.gpsimd.collective_compute(
                            "ReduceScatter",
                            mybir.AluOpType.add,
                            replica_groups=[list(range(N_CORES))],
                            ins=[partial[qc][:, :].opt()],
                            outs=[rs_out[qc][:].opt()],
                        )
                        # bf16 shard -> SBUF -> f32 cast -> output
                        ob = oup.tile([128, D], BF16, tag="ob", name="ob")
                        nc.gpsimd.dma_start(ob[:], rs_out[qc][:])
                        of = oup.tile([128, D], F32, tag="of", name="of")
                        nc.vector.tensor_copy(of[:], ob[:])
                        nc.gpsimd.dma_start(out_ext[qc, :, :], of[:])

                    ops = [work(pt, dh) for pt in range(4) for dh in range(2)]
                    if i == 1:
                        ops.append(rs)
                    return ops

                # --- step machinery ---
                s_tiles = {}
                p_tiles = {}
                cur_acc = {}

                def decode(m):
                    return m // 64, (m // 32) % 2, m % 32

                def emit_scores(m):
                    qc, i, kt = decode(m)
                    qsl = slice(qc * 1024 + i * 512, qc * 1024 + i * 512 + 512)
                    ksl = slice(kt * 128, (kt + 1) * 128)
                    s = scp.tile([128, 1024], F32, tag="s", name=f"s{m}")
                    nc.tensor.matmul(
                        s[:, 0:512], KT_sb[0:DK, ksl], QT_sb[0:DK, qsl],
                        start=True, stop=True,
                    )
                    nc.tensor.matmul(
                        s[:, 512:1024], KT_sb[DK:DH, ksl], QT_sb[DK:DH, qsl],
                        start=True, stop=True,
                    )
                    s_tiles[m] = s

                def emit_exp(m):
                    p = pp.tile([128, 1024], BF16, tag="p", name=f"p{m}")
                    nc.scalar.activation(p[:], s_tiles.pop(m)[:], Exp, scale=0.125)
                    p_tiles[m] = p

                def normalize(qc, i):
                    a0, a1 = cur_acc["a0"], cur_acc["a1"]
                    nc.vector.tensor_copy(den0_sb[:], a0[DK : DK + 1, :])
                    nc.vector.tensor_copy(den1_sb[:], a1[DK : DK + 1, :])
                    nc.vector.reciprocal_approx_fast(rec0_sb[:], den0_sb[:])
                    nc.vector.reciprocal_approx_fast(rec1_sb[:], den1_sb[:])
                    nc.vector.tensor_copy(rec0b_sb[:], rec0_sb[:])
                    nc.vector.tensor_copy(rec1b_sb[:], rec1_sb[:])
                    rb = mpp.tile([128, 512], F32, tag="mp", name=f"rb{qc}{i}")
                    nc.tensor.matmul(
                        rb[0:DK, :], ones64[:], rec0b_sb[:], start=True, stop=True
                    )
                    nc.tensor.matmul(
                        rb[DK:DH, :], ones64[:], rec1b_sb[:], start=True, stop=True
                    )
                    rbs = rbp.tile([128, 512], F32, tag="rb", name="rbs")
                    nc.vector.tensor_copy(rbs[:], rb[:])
                    aT = atp.tile([128, 512], BF16, tag="at", name="aT")
                    nc.vector.tensor_mul(aT[0:DK, :], a0[0:DK, :], rbs[0:DK, :])
                    nc.vector.tensor_mul(aT[DK:DH, :], a1[0:DK, :], rbs[DK:DH, :])
                    oinj.extend(outproj(qc, i, aT))

                def emit_attnv(m):
                    qc, i, kt = decode(m)
                    if kt == 0:
                        cur_acc["a0"] = accp.tile(
                            [DK + 1, 512], F32, tag="a0", name=f"a0_{m}"
                        )
                        cur_acc["a1"] = accp.tile(
                            [DK + 1, 512], F32, tag="a1", name=f"a1_{m}"
                        )
                    p = p_tiles.pop(m)
                    nc.tensor.matmul(
                        cur_acc["a0"][:], vh_sb[:, kt, 0, :], p[:, 0:512],
                        start=(kt == 0), stop=(kt == NKT - 1),
                    )
                    nc.tensor.matmul(
                        cur_acc["a1"][:], vh_sb[:, kt, 1, :], p[:, 512:1024],
                        start=(kt == 0), stop=(kt == NKT - 1),
                    )
                    if kt == NKT - 1:
                        normalize(qc, i)

                # --- main loop ---
                next_av = 0
                emit_scores(0)
                for m in range(N_STEP):
                    if m + 1 < N_STEP:
                        emit_scores(m + 1)
                    emit_exp(m)
                    # drain injected projection work
                    ndrain = 4 if m < 12 else (2 if m < 110 else 0)
                    for _ in range(ndrain):
                        if inj:
                            inj.popleft()()
                    if not inj or m >= 60:
                        for _ in range(2):
                            if oinj:
                                oinj.popleft()()
                    # attnV with lag
                    lag = 12 if m < 64 else 3
                    budget = 2
                    while next_av <= m - lag and budget > 0:
                        emit_attnv(next_av)
                        next_av += 1
                        budget -= 1
                while next_av < N_STEP:
                    emit_attnv(next_av)
                    next_av += 1
                while inj:
                    inj.popleft()()
                while oinj:
                    oinj.popleft()()

    nc.compile()
    return nc


_NC = {}


def _get_nc(with_bias=False, with_mask=False):
    key = (with_bias, with_mask)
    if key not in _NC:
        _NC[key] = _build(with_bias, with_mask)
    return _NC[key]


def make_in_maps(x, Wq, bq, Wk, bk, Wv, bv, Wo, bo, attention_mask=None):
    xT = np.ascontiguousarray(x[0].T).astype(NP_BF16)  # [D, S]
    WqT = np.ascontiguousarray(Wq.T).astype(NP_BF16)  # [D_in, d_out]
    WkT = np.ascontiguousarray(Wk.T).astype(NP_BF16)
    WvT = np.ascontiguousarray(Wv.T).astype(NP_BF16)
    WoT = np.ascontiguousarray(Wo.T).astype(NP_BF16)  # [d_in, d_out]

    with_mask = attention_mask is not None
    if with_mask:
        maskT = np.ascontiguousarray(
            attention_mask.reshape(NKT, 128).T
        ).astype(NP_BF16)

    in_maps = []
    for c in range(N_CORES):
        csl = slice(c * DH, (c + 1) * DH)
        m = {
            "xT": xT,
            "wqT": np.ascontiguousarray(WqT[:, csl]),
            "wkT": np.ascontiguousarray(WkT[:, csl]),
            "wvT": np.ascontiguousarray(WvT[:, csl]),
            "woT": np.ascontiguousarray(WoT[csl, :]),
            "bq": np.ascontiguousarray(bq[None, csl]).astype(NP_BF16),
            "bk": np.ascontiguousarray(bk[None, csl]).astype(NP_BF16),
            "bv": np.ascontiguousarray(bv[None, csl]).astype(NP_BF16),
            # bo must be added exactly once across the ReduceScatter sum
            "bo": (bo[None, :] if c == 0 else np.zeros((1, D), np.float32)).astype(
                NP_BF16
            ),
        }
        if with_mask:
            m["maskT"] = maskT
        in_maps.append(m)
    return in_maps


def assemble_output(results):
    out = np.empty((S, D), np.float32)
    rows = 1024 // N_CORES  # 128
    for c in range(N_CORES):
        o = np.asarray(results[c]["out"])  # [4, 128, D]
        for qc in range(N_QC):
            r0 = qc * 1024 + c * rows
            out[r0 : r0 + rows] = o[qc]
    return out[None, :, :]


def kernel(x, attention_mask, Wq, bq, Wk, bk, Wv, bv, Wo, bo):
    x = np.asarray(x, dtype=np.float32)
    Wq, Wk, Wv, Wo = (np.asarray(w, dtype=np.float32) for w in (Wq, Wk, Wv, Wo))
    bq, bk, bv, bo = (np.asarray(b, dtype=np.float32) for b in (bq, bk, bv, bo))
    mask = np.asarray(attention_mask).reshape(-1)

    with_bias = any(np.any(b) for b in (bq, bk, bv, bo))
    with_mask = not np.all(mask != 0)
    in_maps = make_in_maps(
        x, Wq, bq, Wk, bk, Wv, bv, Wo, bo,
        attention_mask=(mask.astype(np.float32) if with_mask else None),
    )
    nc = _get_nc(with_bias, with_mask)
    res = run_bass_kernel_spmd(nc, in_maps, list(range(N_CORES)))
    return assemble_output(res.results)
